# revision 33
# baseline (speedup 1.0000x reference)
"""KNN (K=1, euclidean) Trainium2 kernel — fp8 DoubleRow edition.

Strategy
--------
Shard the 4096 y-rows across 8 NeuronCores (512 each); replicate x.
Per core, for each 128-row x tile (32 tiles):
  PSUM[128,512] = sum_k q8(2x)^T_k @ q8(y)^T_k   (fp8 e4m3 DoubleRow
                  matmuls, 2 k-tiles per instruction, 2x PE rate, fp32
                  accum) + aug matmul adding -|y|^2 (5 power-of-2-scaled
                  fp8 channels, residual < 0.05)
  => ps[i,j] = u_ij ~= 2 x_i.y_j - |y_j|^2 ; argmax_j u == argmin_j dist
  DVE (3 passes):
    tensor_reduce(max)           -> umax (row max of ps; -umax = t_min)
    scalar_tensor_tensor         -> dj2 = (ps - umax) - j*2^-10
    tensor_reduce(max)           -> pj = max_j dj2 = -argmax_j * 2^-10
  ACT (near-tie count, off the DVE critical path):
    activation(Copy)             -> bias = BAND - umax
    activation(Sign, accum)      -> craw = sum_j sign(ps_j - umax + BAND)
Host: decodes per-core candidates (j = -pj*2^10), recomputes candidate
distances exactly in fp64, and for rows whose winner has near-ties resolves
exactly with per-suspect-core fp64 GEMMs; applies sqrt / buffer
scatter-update semantics of the reference.
"""

import numpy as np
import ml_dtypes

P = 128          # partitions
KT = 24          # k tiles (3072 / 128)
NJ = 512         # y rows per core
MT = 32          # x tiles (4096 / 128)
NCORES = 8
D = 3072
B = 4096
EPS = 2.0 ** -10   # argmax packing step (survives fp32 ulp at |u|~3500)
BAND = 42.0        # near-tie band in u units (fp8 + 3-dropped-dims, std ~5.4)
NAUG = 3           # k-dims stolen for the -|y|^2 channels (dims D-NAUG..D-1)
AUGW = [64.0, 8.0, 0.5]  # exact power-of-2 channel weights
F8 = ml_dtypes.float8_e4m3

_CACHE = {}


def build_nc(mt=MT):
    import concourse.bacc as bacc
    import concourse.mybir as mybir
    import concourse.tile as tile

    f8 = mybir.dt.float8e4
    f32 = mybir.dt.float32
    bf16 = mybir.dt.bfloat16
    DR = mybir.MatmulPerfMode.DoubleRow

    nc = bacc.Bacc("TRN2", target_bir_lowering=False, debug=False)

    xw = nc.dram_tensor("xw", (mt, P, KT, P), f8, kind="ExternalInput")
    yw = nc.dram_tensor("yw", (P, KT, NJ), f8, kind="ExternalInput")
    iote = nc.dram_tensor("iote", (P, NJ), f32, kind="ExternalInput")
    res = nc.dram_tensor("res", (P, 3 * mt), f32, kind="ExternalOutput")

    with tile.TileContext(nc) as tc:
        with (
            tc.tile_pool(name="const", bufs=1) as cpool,
            tc.tile_pool(name="xpool", bufs=4) as xpool,
            tc.tile_pool(name="work", bufs=3) as wpool,
            tc.tile_pool(name="psum", bufs=4, space="PSUM") as ppool,
        ):
            # x tile 0 issued before the y preload so the first matmul's
            # inputs land promptly (each queue serializes ~600ns per issue)
            x0_sb = xpool.tile((P, KT, P), f8, tag="xw")
            nc.sync.dma_start(x0_sb[:], xw[0])
            # y preload as 12 per-k-pair tiles (full-tile contiguous rhs for
            # the matmuls — mid-tile rhs slices slow the PE ~20%), issues
            # round-robined across three queues to land in need order
            y_tiles = []
            for kk in range(KT // 2):
                yt = cpool.tile((P, 2, NJ), f8, tag=f"y{kk}")
                eng = (nc.gpsimd, nc.scalar, nc.sync)[kk % 3]
                eng.dma_start(yt[:], yw[:, 2 * kk:2 * kk + 2, :])
                y_tiles.append(yt)
            iote_sb = cpool.tile((P, NJ), f32)
            nc.gpsimd.dma_start(iote_sb[:], iote[:])
            res_sb = cpool.tile((P, 3 * mt), f32)

            for m in range(mt):
                if m == 0:
                    x_sb = x0_sb
                else:
                    x_sb = xpool.tile((P, KT, P), f8, tag="xw")
                    nc.sync.dma_start(x_sb[:], xw[m])
                ps = ppool.tile((P, NJ), f32, tag="ps")
                for kk in range(KT // 2):
                    nc.tensor.matmul(
                        ps[:],
                        x_sb[:, 2 * kk:2 * kk + 2, :],
                        y_tiles[kk][:],
                        start=(kk == 0), stop=(kk == KT // 2 - 1),
                        perf_mode=DR,
                    )

                umax = res_sb[:, 3 * m:3 * m + 1]
                nc.vector.tensor_reduce(umax, ps[:],
                                        axis=mybir.AxisListType.X,
                                        op=mybir.AluOpType.max)
                # dj2 = (ps - umax) - j*EPS; row max = -argmax*EPS
                # (exact at the winner: ps - umax == 0 there)
                dj2 = wpool.tile((P, NJ), f32, tag="dj2")
                nc.vector.scalar_tensor_tensor(
                    out=dj2[:], in0=ps[:], scalar=umax, in1=iote_sb[:],
                    op0=mybir.AluOpType.subtract, op1=mybir.AluOpType.subtract,
                )
                nc.vector.tensor_reduce(res_sb[:, 3 * m + 1:3 * m + 2], dj2[:],
                                        axis=mybir.AxisListType.X,
                                        op=mybir.AluOpType.max)
                # near-tie count on the scalar engine (DVE stays 3 passes)
                bias_c = wpool.tile((P, 1), f32, tag="bias")
                nc.scalar.activation(
                    bias_c[:], umax,
                    mybir.ActivationFunctionType.Copy,
                    bias=float(BAND), scale=-1.0,
                )
                msk = wpool.tile((P, NJ), bf16, tag="msk")
                nc.scalar.activation(
                    msk[:], ps[:],
                    mybir.ActivationFunctionType.Sign,
                    bias=bias_c[:], scale=1.0,
                    accum_out=res_sb[:, 3 * m + 2:3 * m + 3],
                )
            nc.sync.dma_start(res[:], res_sb[:])
    return nc


def _fp8(a):
    return np.asarray(a, np.float32).astype(F8)


def _y2_channels(neg_y2):
    """Split -|y|^2 (fp64, ~[-3500,-2500]) into NAUG fp8 channels with exact
    power-of-2 weights so that sum_r w_r * fp8(ch_r) ~= -|y|^2 (|res|<0.6)."""
    r = neg_y2.copy()
    chans = []
    for w in AUGW:
        a8 = _fp8(r / w)
        chans.append(a8)
        r = r - w * a8.astype(np.float64)
    return chans, r


def make_inputs(x, y):
    """Host-side input prep: per-core in_maps (shared x weights, per-core y).

    The last NAUG dims (D-NAUG..D-1) are dropped from the device computation;
    their k-tile partition rows carry the -|y|^2 aug channels instead (x side
    holds the constant channel weight, y side the channel values). The host
    postprocess always re-scores candidates in exact fp64 over all D dims, so
    the dropped dims only widen the device's error band (covered by BAND)."""
    xs = _fp8(2.0 * np.asarray(x, np.float32))
    # xw[mt, p, k, m] = q8(2x)[mt*128+m, k*128+p]
    xw = np.ascontiguousarray(
        xs.reshape(MT, P, KT, P).transpose(0, 3, 2, 1))
    for r, w in enumerate(AUGW):
        xw[:, P - NAUG + r, KT - 1, :] = np.float32(w)
    iote = np.broadcast_to(
        (np.arange(NJ, dtype=np.float64) * EPS).astype(np.float32), (P, NJ)
    ).copy()

    y64 = np.asarray(y).astype(np.float64)
    y2g = np.sum(y64 * y64, axis=1)  # fp64 row norms of full y

    in_maps = []
    for c in range(NCORES):
        yc8 = _fp8(y[c * NJ:(c + 1) * NJ])
        # yw[p, k, n] = q8(y_c)[n, k*128+p]
        yw = np.ascontiguousarray(yc8.reshape(NJ, KT, P).transpose(2, 1, 0))
        chans, rres = _y2_channels(-y2g[c * NJ:(c + 1) * NJ])
        assert np.abs(rres).max() < 0.6, np.abs(rres).max()
        for r, ch in enumerate(chans):
            yw[P - NAUG + r, KT - 1, :] = ch
        in_maps.append({"xw": xw, "yw": yw, "iote": iote})
    return in_maps, y2g


def decode_core(res_c, mt=MT):
    """res_c [128, 3*mt] -> (tmin[B], jloc[B], cnt[B], anom[B]) in x-row order."""
    umax = res_c[:, 0::3].T.reshape(-1).astype(np.float64)
    pj = res_c[:, 1::3].T.reshape(-1).astype(np.float64)
    craw = res_c[:, 2::3].T.reshape(-1).astype(np.float64)
    cnt = (craw + NJ) / 2.0            # sign-sum -> #{>} + #{=}/2
    tmin = -umax                       # t = -u
    jf = -pj / EPS
    jloc = np.rint(jf).astype(np.int64)
    anom = (np.abs(jf - jloc) > 0.35) | (jloc < 0) | (jloc >= NJ)
    jloc = np.clip(jloc, 0, NJ - 1)
    return tmin, jloc, cnt, anom


def postprocess(results, x, y, y2g, min_dists, nn_indices,
                x_idx_start, y_idx_start):
    nb = x.shape[0]
    x64 = np.asarray(x).astype(np.float64)
    y64 = np.asarray(y).astype(np.float64)
    x2 = np.sum(x64 * x64, axis=1)

    tmins = np.empty((NCORES, nb))
    jglob = np.empty((NCORES, nb), np.int64)
    cnts = np.empty((NCORES, nb))
    anoms = np.zeros(nb, bool)
    for c in range(NCORES):
        tm, jl, cn, an = decode_core(np.asarray(results[c]["res"]))
        tmins[c] = tm
        jglob[c] = c * NJ + jl
        cnts[c] = cn
        anoms |= an

    # exact fp64 t for every per-core candidate
    tex = np.empty((NCORES, nb))
    for c in range(NCORES):
        yj = y64[jglob[c]]
        tex[c] = y2g[jglob[c]] - 2.0 * np.einsum("ij,ij->i", x64, yj)

    best = tex.min(axis=0)
    # exact cross-core tie on best value -> pick smallest j
    tie = tex <= best[None, :]
    jtie = np.where(tie, jglob, np.iinfo(np.int64).max)
    jbest = jtie.min(axis=0)

    # suspect cores: near-tie inside the core AND device min close to best.
    # tmins (device) is used, not tex: decode aliasing (two dev values within
    # 512*EPS) can make the candidate j meaningless, but tmin_dev is sound
    # (tmin_dev(c*) <= truemin + E1 <= best + E1, single-pair error bound).
    sus = (cnts >= 1.4) & (tmins <= best[None, :] + BAND)
    flag = sus.any(axis=0) & ~anoms

    # resolve flagged rows with per-core fp64 GEMMs over suspect cores only
    if flag.any():
        for c in range(NCORES):
            rows = np.where(flag & sus[c])[0]
            if not rows.size:
                continue
            yc = y64[c * NJ:(c + 1) * NJ]
            tall = y2g[None, c * NJ:(c + 1) * NJ] - 2.0 * (x64[rows] @ yc.T)
            jt = np.argmin(tall, axis=1)           # first occurrence = min j
            tv = tall[np.arange(rows.size), jt]
            jg = c * NJ + jt
            better = (tv < best[rows]) | ((tv == best[rows]) & (jg < jbest[rows]))
            upd = rows[better]
            best[upd] = tv[better]
            jbest[upd] = jg[better]

    # anomalous rows (decode failure): full-row exact recompute
    frows = np.where(anoms)[0]
    if frows.size:
        CH = 256
        for s in range(0, frows.size, CH):
            rr = frows[s:s + CH]
            tall = y2g[None, :] - 2.0 * (x64[rr] @ y64.T)
            jt = np.argmin(tall, axis=1)
            best[rr] = tall[np.arange(rr.size), jt]
            jbest[rr] = jt

    d2 = x2 + best
    new_min = np.sqrt(np.maximum(d2, 0.0)).astype(np.float32)

    md = np.array(min_dists, dtype=np.float32, copy=True)
    ni = np.array(nn_indices, dtype=np.int32, copy=True)
    n = md.shape[0]
    s = int(np.asarray(x_idx_start))
    s = max(0, min(s, n - nb))  # dynamic_update_slice clamp semantics
    md[s:s + nb] = np.minimum(new_min, md[s:s + nb])
    ni[s:s + nb] = (jbest.astype(np.int64)
                    + int(np.asarray(y_idx_start))).astype(np.int32)
    return md, ni


def _get_nc():
    if "nc" not in _CACHE:
        nc = build_nc()
        nc.compile()
        _CACHE["nc"] = nc
    return _CACHE["nc"]


def run_device(in_maps, trace=False, **kw):
    from concourse.bass_utils import run_bass_kernel_spmd
    nc = _get_nc()
    return run_bass_kernel_spmd(nc, in_maps, list(range(NCORES)),
                                trace=trace, **kw)


def kernel(x, y, min_dists, nn_indices, x_idx_start, y_idx_start):
    x = np.asarray(x)
    y = np.asarray(y)
    in_maps, y2g = make_inputs(x, y)
    br = run_device(in_maps, trace=False)
    return postprocess(br.results, x, y, y2g, min_dists, nn_indices,
                       x_idx_start, y_idx_start)
